# revision 2
# baseline (speedup 1.0000x reference)
"""Job2vec embedding lookup + output projection on 8 TRN2 NeuronCores.

Math: u = W1[ids] @ W2   (ids [2048], W1 [100000,128], W2 [128,100000])

Sharding: W2 is split along its vocab axis into 8 shards of 12500 columns;
every core gathers the full h = W1[ids] (tiny: 1 MB) and computes the full
batch against its own W2 shard, writing out [2048, 12500]. The host
concatenates the 8 shards along axis 1. Output write (819 MB total,
102 MB/core) dominates -> memory-bound as expected.

Per-core device pipeline:
  1. DMA ids (host-prearranged [128, 16] int32) and the W2 shard into SBUF.
  2. Indirect-DMA gather 16x [128, 128] rows of W1 (bf16).
  3. PE-transpose each gathered tile into hT [128(dim), 2048(batch)].
  4. For each of 16 batch tiles: 25 matmuls hT_tile.T @ W2s[:, n*500:...]
     into PSUM (f32 accum), copy PSUM->SBUF row buffer, one 6.4 MB DMA out.
"""

import numpy as np
import ml_dtypes

B = 2048  # batch
V = 100000  # vocab
D = 128  # embedding dim
NCORES = 8
VS = V // NCORES  # 12500 vocab columns per core
MT = B // 128  # 16 batch tiles
NTILE = 500  # matmul free-dim tile (one PSUM bank of f32)
NT = VS // NTILE  # 25 vocab tiles per core

_CACHED_NC = None


def _build_nc():
    import concourse.bacc as bacc
    import concourse.bass as bass
    import concourse.mybir as mybir
    import concourse.tile as tile
    from concourse.masks import make_identity

    CDT = mybir.dt.bfloat16
    ODT = mybir.dt.float32

    nc = bacc.Bacc("TRN2", target_bir_lowering=False, debug=False)

    ids = nc.dram_tensor("ids", [128, MT], mybir.dt.int32, kind="ExternalInput")
    w1 = nc.dram_tensor("w1", [V, D], CDT, kind="ExternalInput")
    w2s = nc.dram_tensor("w2s", [D, VS], CDT, kind="ExternalInput")
    out = nc.dram_tensor("out", [B, VS], ODT, kind="ExternalOutput")

    with tile.TileContext(nc) as tc:
        with (
            tc.tile_pool(name="const", bufs=1) as cpool,
            tc.tile_pool(name="gather", bufs=4) as gpool,
            tc.tile_pool(name="tpsum", bufs=2, space="PSUM") as tpsum,
            tc.tile_pool(name="mmpsum", bufs=4, space="PSUM") as mpsum,
            tc.tile_pool(name="outbuf", bufs=2) as opool,
        ):
            identity = cpool.tile([128, 128], CDT)
            make_identity(nc, identity[:])

            ids_sb = cpool.tile([128, MT], mybir.dt.int32)
            nc.sync.dma_start(out=ids_sb[:], in_=ids[:])

            w2_sb = cpool.tile([D, VS], CDT)
            nc.sync.dma_start(out=w2_sb[:], in_=w2s[:])

            # Gather h rows then transpose into hT [dim, batch].
            hT = cpool.tile([D, B], CDT)
            for j in range(MT):
                h_tile = gpool.tile([128, D], CDT, tag="h")
                nc.gpsimd.indirect_dma_start(
                    out=h_tile[:],
                    out_offset=None,
                    in_=w1[:],
                    in_offset=bass.IndirectOffsetOnAxis(ap=ids_sb[:, j : j + 1], axis=0),
                )
                pt = tpsum.tile([128, 128], CDT)
                nc.tensor.transpose(out=pt[:], in_=h_tile[:], identity=identity[:])
                nc.vector.tensor_copy(out=hT[:, j * 128 : (j + 1) * 128], in_=pt[:])

            for m in range(MT):
                ob = opool.tile([128, VS], ODT, tag="ob")
                for n in range(NT):
                    ps = mpsum.tile([128, NTILE], mybir.dt.float32, tag="ps")
                    nc.tensor.matmul(
                        out=ps[:],
                        lhsT=hT[:, m * 128 : (m + 1) * 128],
                        rhs=w2_sb[:, n * NTILE : (n + 1) * NTILE],
                        start=True,
                        stop=True,
                    )
                    # Split PSUM->SBUF copies between DVE and ACT.
                    if n % 2 == 0:
                        nc.vector.tensor_copy(
                            out=ob[:, n * NTILE : (n + 1) * NTILE], in_=ps[:]
                        )
                    else:
                        nc.scalar.copy(out=ob[:, n * NTILE : (n + 1) * NTILE], in_=ps[:])
                nc.sync.dma_start(out=out[m * 128 : (m + 1) * 128, :], in_=ob[:])

    nc.finalize()
    return nc


def _get_nc():
    global _CACHED_NC
    if _CACHED_NC is None:
        _CACHED_NC = _build_nc()
    return _CACHED_NC


def _make_in_maps(inputs):
    ids = np.asarray(inputs["inputs"]).reshape(B).astype(np.int32)
    # Device wants ids as [128, MT] with ids_dev[p, j] = ids[j*128 + p].
    ids_dev = np.ascontiguousarray(ids.reshape(MT, 128).T)
    w1 = np.asarray(inputs["W1"], dtype=np.float32).astype(ml_dtypes.bfloat16)
    w2 = np.asarray(inputs["W2"], dtype=np.float32)
    in_maps = []
    for c in range(NCORES):
        w2s = np.ascontiguousarray(w2[:, c * VS : (c + 1) * VS]).astype(
            ml_dtypes.bfloat16
        )
        in_maps.append({"ids": ids_dev, "w1": w1, "w2s": w2s})
    return in_maps


def _run(inputs, trace=False, tmpdir=None):
    from concourse.bass_utils import run_bass_kernel_spmd

    nc = _get_nc()
    in_maps = _make_in_maps(inputs)
    res = run_bass_kernel_spmd(
        nc, in_maps, list(range(NCORES)), trace=trace, tmpdir=tmpdir
    )
    out = np.concatenate(
        [res.results[c]["out"] for c in range(NCORES)], axis=1
    ).astype(np.float32)
    return out, res


def kernel(**inputs) -> np.ndarray:
    out, _ = _run(inputs)
    return out
